# revision 56
# baseline (speedup 1.0000x reference)
"""Trainium2 Bass kernel for nn_AttentionUnit (B=4, S=2048, D=1024, H=16).

Sharding: 8 cores = 4 batches x 2 head-groups (Megatron column/row split).
Each core computes, for its (batch, 8-head half):
  Q^T,K^T = (Wq/Wk half)^T-proj of inputs   [dh=512 on partitions, seq free]
  V       = natural [seq, dh] per head, with a ones column on each side
  S^T     = K @ Q^T / 8 per head-pair tile (causal blocks skipped,
            padding via exp bias shared across the pair)
  P^T     = exp(S^T)  (unnormalized, bf16, one activation per (pair, kt))
  O^T     = V_aug^T @ P^T  -> even heads rows 0..64 (denom at 64),
            odd heads rows 63..127 (denom at 63): attn lands at its final
            partition offset, no cross-partition DMA.
  attn^T  = O^T * recip(denom) -> AT [512, seq] bf16 on chip
  partial = attn @ Wo_half     [seq, 1024] fp32 -> host (direct PSUM->DRAM)
Host: out[b] = partial[2b] + partial[2b+1] + bo (+ bv@Wo).

Schedule: warmup matmuls ramp the PE clock during the initial weight DMA;
per q-block emission order is B(qb), A(qb+1), C(qb) so the output
projection never waits on the last head-pair's normalize chain.
"""

import sys

sys.path.insert(0, "/opt/trn_rl_repo")

import numpy as np
import ml_dtypes

S = 2048
D = 1024
P = 128
DH = 64          # head dim
HPC = 8          # heads per core
DHH = 512        # dh per core (8 heads * 64)
QB = 512         # q block
NQB = S // QB    # 4
DMC = D // P     # 8 dmodel chunks
NT = S // P      # 16 k tiles
NEG_CAUSAL = -1.0e12   # added pre-scale (scale=0.125 applied inside exp)
NEG_PAD = -1.0e9       # added post-scale (exp bias)
NWARM = 5              # PE clock-ramp matmuls during initial DMA

_CACHE = {}


def _build_program(kcap=NT, warm=NWARM):
    import concourse.bass as bass
    import concourse.tile as tile
    from concourse import bacc, mybir

    f32 = mybir.dt.float32
    bf16 = mybir.dt.bfloat16
    ADD = mybir.AluOpType.add
    MUL = mybir.AluOpType.mult
    EXP = mybir.ActivationFunctionType.Exp

    nc = bacc.Bacc("TRN2", target_bir_lowering=False, debug=False)

    qt_d = nc.dram_tensor("qt", [D, S], bf16, kind="ExternalInput")
    kt_d = nc.dram_tensor("kt", [D, S], bf16, kind="ExternalInput")
    vt_d = nc.dram_tensor("vt", [D, S], bf16, kind="ExternalInput")
    wq_d = nc.dram_tensor("wq", [D, DHH], bf16, kind="ExternalInput")
    wk_d = nc.dram_tensor("wk", [D, DHH], bf16, kind="ExternalInput")
    wv_d = nc.dram_tensor("wv", [D, DHH], bf16, kind="ExternalInput")
    wo_d = nc.dram_tensor("wo", [DHH, D], bf16, kind="ExternalInput")
    padb_d = nc.dram_tensor("padb", [P, NT], f32, kind="ExternalInput")
    mask_d = nc.dram_tensor("mask", [P, 2, P], bf16, kind="ExternalInput")
    bq_d = nc.dram_tensor("bq", [P, 4], f32, kind="ExternalInput")
    bk_d = nc.dram_tensor("bk", [P, 4], f32, kind="ExternalInput")
    out_d = nc.dram_tensor("outp", [S, D], bf16, kind="ExternalOutput")

    qt_r = qt_d.rearrange("(c p) s -> p c s", p=P)
    kt_r = kt_d.rearrange("(c p) s -> p c s", p=P)
    vt_r = vt_d.rearrange("(c p) s -> p c s", p=P)

    with tile.TileContext(nc) as tc:
        with (
            tc.tile_pool(name="const", bufs=1) as constp,
            tc.tile_pool(name="inp", bufs=2) as inp,
            tc.tile_pool(name="probs", bufs=4) as probsp,
            tc.tile_pool(name="small", bufs=2) as smallp,
            tc.tile_pool(name="pvs", bufs=2) as pvsp,
            tc.tile_pool(name="osb", bufs=2) as osbp,
            tc.tile_pool(name="psSC", bufs=3, space="PSUM") as psSC,
            tc.tile_pool(name="psPV", bufs=2, space="PSUM") as psPV,
        ):
            # ---- warmup: ramp the PE clock while weights stream in ----
            wa = constp.tile([P, P], bf16, tag="wa")
            wb = constp.tile([P, QB], bf16, tag="wb")
            nc.vector.memset(wa[:], 0.0)
            nc.vector.memset(wb[:], 0.0)
            wps = psSC.tile([P, 2, QB], f32, tag="sc")
            for i in range(warm):
                nc.tensor.matmul(wps[:, 0, :], wa[:], wb[:], start=True, stop=True)
                if i == 1:
                    # pull the Exp table load off the critical path
                    wact = constp.tile([P, DH], bf16, tag="wact")
                    nc.scalar.activation(wact[:], wps[:, 0, 0:DH], EXP, scale=1.0)

            # ---- persistent SBUF tensors ----
            wq_sb = constp.tile([P, DMC, DHH], bf16, tag="wq")
            wk_sb = constp.tile([P, DMC, DHH], bf16, tag="wk")
            wv_sb = constp.tile([P, DMC, DHH], bf16, tag="wv")
            wo_sb = constp.tile([P, 4, D], bf16, tag="wo")
            QT = constp.tile([P, 4, S], bf16, tag="QT")
            KT = constp.tile([P, 4, S], bf16, tag="KT")
            V = constp.tile([P, NT, HPC, DH + 1], bf16, tag="V")
            AT = constp.tile([P, 4, S], bf16, tag="AT")
            padb = constp.tile([P, NT], f32, tag="padb")
            mask = constp.tile([P, 2, P], bf16, tag="mask")
            bq_sb = constp.tile([P, 4], f32, tag="bq")
            bk_sb = constp.tile([P, 4], f32, tag="bk")

            # qb0 inputs and weights stream in matched mc-chunks, with the
            # accumulating projection matmuls emitted right behind each chunk
            # pair so the PE starts after ~300KB instead of the full 8MB.
            wq_r = wq_d.rearrange("(c p) m -> p c m", p=P)
            wk_r = wk_d.rearrange("(c p) m -> p c m", p=P)
            wv_r = wv_d.rearrange("(c p) m -> p c m", p=P)
            qin0 = inp.tile([P, DMC, QB], bf16, tag="qin")
            kin0 = inp.tile([P, DMC, QB], bf16, tag="kin")
            vin0 = inp.tile([P, DMC, QB], bf16, tag="vin")
            nc.sync.dma_start(padb[:], padb_d[:])
            nc.sync.dma_start(mask[:], mask_d[:])
            nc.sync.dma_start(bq_sb[:], bq_d[:])
            nc.sync.dma_start(bk_sb[:], bk_d[:])
            # ones column for the softmax denominator
            nc.vector.memset(V[:, :, :, DH : DH + 1], 1.0)

            ins = [None] * (NQB + 1)
            ins[0] = (qin0, kin0, vin0)

            def stream_proj(w_sb, w_r, in_sb, in_r, operands_of_mc):
                """DMA-chase projection: per-mc chunk DMAs immediately
                followed by the 4 accumulating matmuls that consume them.
                Returns the two PSUM tiles holding c-chunks (0,1) and (2,3)."""
                t01 = psSC.tile([P, 2, QB], f32, tag="sc")
                t23 = psSC.tile([P, 2, QB], f32, tag="sc")
                tiles = [t01, t23]
                for mc in range(DMC):
                    nc.sync.dma_start(w_sb[:, mc, :], w_r[:, mc, :])
                    nc.sync.dma_start(in_sb[:, mc, :], in_r[:, mc, 0:QB])
                    for c in range(4):
                        lhsT, rhs = operands_of_mc(w_sb, in_sb, mc, c)
                        nc.tensor.matmul(
                            tiles[c // 2][:, c % 2, :], lhsT, rhs,
                            start=(mc == 0), stop=(mc == DMC - 1),
                        )
                return tiles

            def phase_a0():
                tq = stream_proj(
                    wq_sb, wq_r, qin0, qt_r,
                    lambda w, i, mc, c: (w[:, mc, c * P : (c + 1) * P], i[:, mc, :]),
                )
                for c in range(4):
                    nc.scalar.add(
                        QT[:, c, 0:QB], tq[c // 2][:, c % 2, :], bq_sb[:, c : c + 1]
                    )
                tk = stream_proj(
                    wk_sb, wk_r, kin0, kt_r,
                    lambda w, i, mc, c: (w[:, mc, c * P : (c + 1) * P], i[:, mc, :]),
                )
                for c in range(4):
                    nc.scalar.add(
                        KT[:, c, 0:QB], tk[c // 2][:, c % 2, :], bk_sb[:, c : c + 1]
                    )
                tv = stream_proj(
                    wv_sb, wv_r, vin0, vt_r,
                    lambda w, i, mc, j: (i[:, mc, j * P : (j + 1) * P], w[:, mc, :]),
                )
                for j in range(4):
                    nc.vector.tensor_copy(
                        V[:, j, :, 0:DH],
                        tv[j // 2][:, j % 2, :].rearrange("p (h d) -> p h d", d=DH),
                    )

            def load_inputs(qb):
                q0 = qb * QB
                qin = inp.tile([P, DMC, QB], bf16, tag="qin")
                kin = inp.tile([P, DMC, QB], bf16, tag="kin")
                vin = inp.tile([P, DMC, QB], bf16, tag="vin")
                nc.sync.dma_start(qin[:], qt_r[:, :, q0 : q0 + QB])
                nc.sync.dma_start(kin[:], kt_r[:, :, q0 : q0 + QB])
                nc.sync.dma_start(vin[:], vt_r[:, :, q0 : q0 + QB])
                ins[qb] = (qin, kin, vin)

            def phase_a_groups(qb):
                """Projection work for q-block qb as 6 emission closures
                [Q01, K01, Q23, K23, V01, V23] so groups can be interleaved
                at B(qb-1) pair boundaries as PE filler."""
                q0 = qb * QB
                qin, kin, vin = ins[qb]
                kn = min(QB, max(0, kcap * P - q0))

                def qgrp(cp):
                    ps = psSC.tile([P, 2, QB], f32, tag="sc")
                    for jj in range(2):
                        c = 2 * cp + jj
                        for mc in range(DMC):
                            nc.tensor.matmul(
                                ps[:, jj, :],
                                wq_sb[:, mc, c * P : (c + 1) * P],
                                qin[:, mc, :],
                                start=(mc == 0),
                                stop=(mc == DMC - 1),
                            )
                    for jj in range(2):
                        c = 2 * cp + jj
                        nc.scalar.add(
                            QT[:, c, q0 : q0 + QB], ps[:, jj, :],
                            bq_sb[:, c : c + 1],
                        )

                def kgrp(cp):
                    if kn <= 0:
                        return
                    ps = psSC.tile([P, 2, QB], f32, tag="sc")
                    for jj in range(2):
                        c = 2 * cp + jj
                        for mc in range(DMC):
                            nc.tensor.matmul(
                                ps[:, jj, :kn],
                                wk_sb[:, mc, c * P : (c + 1) * P],
                                kin[:, mc, :kn],
                                start=(mc == 0),
                                stop=(mc == DMC - 1),
                            )
                    for jj in range(2):
                        c = 2 * cp + jj
                        nc.scalar.add(
                            KT[:, c, q0 : q0 + kn], ps[:, jj, :kn],
                            bk_sb[:, c : c + 1],
                        )

                def vgrp(jp):
                    if 4 * qb + 2 * jp >= kcap:
                        return
                    ps = psSC.tile([P, 2, QB], f32, tag="sc")
                    for jj in range(2):
                        j = 2 * jp + jj
                        if 4 * qb + j >= kcap:
                            continue
                        for mc in range(DMC):
                            nc.tensor.matmul(
                                ps[:, jj, :],
                                vin[:, mc, j * P : (j + 1) * P],
                                wv_sb[:, mc, :],
                                start=(mc == 0),
                                stop=(mc == DMC - 1),
                            )
                    for jj in range(2):
                        j = 2 * jp + jj
                        kt_i = 4 * qb + j
                        if kt_i >= kcap:
                            continue
                        nc.vector.tensor_copy(
                            V[:, kt_i, :, 0:DH],
                            ps[:, jj, :].rearrange("p (h d) -> p h d", d=DH),
                        )

                return [
                    lambda: qgrp(0), lambda: kgrp(0),
                    lambda: qgrp(1), lambda: kgrp(1),
                    lambda: vgrp(0), lambda: vgrp(1),
                ]

            def phase_b(qb, filler=None):
                """Attention for q-block qb -> AT[:, :, q0:q0+QB]. After
                pairs 0-2, one next-block projection group is emitted as PE
                filler to cover the normalize-chain drain on DVE."""
                q0 = qb * QB
                kt_max = min(4 * qb + 4, kcap)
                for pair in range(4):
                    if filler and pair > 0:
                        filler.pop(0)()
                    h0, h1 = 2 * pair, 2 * pair + 1
                    pv_e = psPV.tile([P, QB], f32, tag="pv")
                    pv_o = psPV.tile([P, QB], f32, tag="pv")

                    def emit_sc(kt):
                        delta = max(0, P * kt - q0)
                        sc = psSC.tile([P, 2, QB], f32, tag="sc")
                        nc.tensor.matmul(
                            sc[:, 0, delta:],
                            KT[0:DH, pair, kt * P : (kt + 1) * P],
                            QT[0:DH, pair, q0 + delta : q0 + QB],
                            start=True, stop=True,
                        )
                        nc.tensor.matmul(
                            sc[:, 1, delta:],
                            KT[DH:P, pair, kt * P : (kt + 1) * P],
                            QT[DH:P, pair, q0 + delta : q0 + QB],
                            start=True, stop=True,
                        )
                        return sc

                    def emit_exp(kt, sc):
                        delta = max(0, P * kt - q0)
                        pr = probsp.tile([P, 2, QB], bf16, tag="probs")
                        nc.scalar.activation(
                            pr[:, :, delta:],
                            sc[:, :, delta:],
                            EXP,
                            bias=padb[:, kt : kt + 1],
                            scale=0.125,
                        )
                        if kt >= 4 * qb:
                            # zero the upper-triangle probs of the diagonal
                            # block (cheap bf16 SBUF multiply, 2x DVE mode)
                            nc.vector.tensor_tensor(
                                pr[:, :, delta : delta + P],
                                pr[:, :, delta : delta + P],
                                mask[:],
                                MUL,
                            )
                        return pr

                    def emit_pv(kt, pr):
                        delta = max(0, P * kt - q0)
                        nc.tensor.matmul(
                            pv_e[0 : DH + 1, delta:],
                            V[:, kt, h0, 0 : DH + 1],
                            pr[:, 0, delta:],
                            start=(kt == 0),
                            stop=(kt == kt_max - 1),
                        )
                        nc.tensor.matmul(
                            pv_o[0 : DH + 1, delta:],
                            V[:, kt, h1, 0 : DH + 1],
                            pr[:, 1, delta:],
                            start=(kt == 0),
                            stop=(kt == kt_max - 1),
                        )

                    # 2-deep software pipeline: the PE stream runs sc(k+2)
                    # ahead of pv(k) so pv's exp wait (and the pv-bank WAR at
                    # pair transitions) is fully covered.
                    prs = {}
                    for kt in range(min(2, kt_max)):
                        prs[kt] = emit_exp(kt, emit_sc(kt))
                    for kt in range(kt_max):
                        if kt + 2 < kt_max:
                            sc = emit_sc(kt + 2)
                            emit_pv(kt, prs.pop(kt))
                            prs[kt + 2] = emit_exp(kt + 2, sc)
                        else:
                            emit_pv(kt, prs.pop(kt))
                    # normalize: copy PSUM out early (releases pv banks), DMA
                    # the two denom rows to lane 0, one approx recip each, two
                    # partition broadcasts, two multiplies; odd half reaches
                    # its upper partitions via one SBUF->SBUF DMA. The very
                    # last pair is chunked so phase C's first output tile can
                    # start as soon as its 128 columns are normalized.
                    cp_e = pvsp.tile([P, QB], f32, tag="pvc")
                    cp_o = pvsp.tile([P, QB], f32, tag="pvc")
                    last = (qb == NQB - 1) and (pair == 3)
                    chunks = (P, P, P, P) if last else (QB,)
                    c0 = 0
                    for w in chunks:
                        nc.vector.tensor_copy(
                            cp_e[0 : DH + 1, c0 : c0 + w], pv_e[0 : DH + 1, c0 : c0 + w]
                        )
                        nc.vector.tensor_copy(
                            cp_o[0 : DH + 1, c0 : c0 + w], pv_o[0 : DH + 1, c0 : c0 + w]
                        )
                        c0 += w
                    x0 = 0
                    for w in chunks:
                        xs = slice(x0, x0 + w)
                        den_e = smallp.tile([1, QB], f32, tag="dene")
                        den_o = smallp.tile([1, QB], f32, tag="deno")
                        nc.sync.dma_start(den_e[0:1, 0:w], cp_e[DH : DH + 1, xs])
                        nc.sync.dma_start(den_o[0:1, 0:w], cp_o[DH : DH + 1, xs])
                        rcp_e = smallp.tile([1, QB], f32, tag="rcpe")
                        rcp_o = smallp.tile([1, QB], f32, tag="rcpo")
                        nc.vector.reciprocal_approx_fast(
                            rcp_e[0:1, 0:w], den_e[0:1, 0:w]
                        )
                        nc.vector.reciprocal_approx_fast(
                            rcp_o[0:1, 0:w], den_o[0:1, 0:w]
                        )
                        rep = smallp.tile([DH, 2, QB], f32, tag="rep")
                        nc.gpsimd.partition_broadcast(rep[:, 0, 0:w], rcp_e[0:1, 0:w])
                        nc.gpsimd.partition_broadcast(rep[:, 1, 0:w], rcp_o[0:1, 0:w])
                        nc.vector.tensor_tensor(
                            AT[0:DH, pair, q0 + x0 : q0 + x0 + w],
                            cp_e[0:DH, xs], rep[:, 0, 0:w], MUL,
                        )
                        tmp = smallp.tile([DH, QB], bf16, tag="tmp")
                        nc.vector.tensor_tensor(
                            tmp[0:DH, 0:w], cp_o[0:DH, xs], rep[:, 1, 0:w], MUL,
                        )
                        nc.sync.dma_start(
                            AT[DH:P, pair, q0 + x0 : q0 + x0 + w], tmp[0:DH, 0:w]
                        )
                        x0 += w

            def phase_c(qb):
                """Output projection for q-block qb -> out rows."""
                for j in range(4):
                    qt_i = 4 * qb + j
                    fin = psSC.tile([P, 2, QB], f32, tag="sc")
                    for half in range(2):
                        for c in range(4):
                            nc.tensor.matmul(
                                fin[:, half, :],
                                AT[:, c, qt_i * P : (qt_i + 1) * P],
                                wo_sb[:, c, half * 512 : half * 512 + 512],
                                start=(c == 0),
                                stop=(c == 3),
                            )
                    osb = osbp.tile([P, D], bf16, tag="osb")
                    if qb == NQB - 1 and j == 3:
                        # final tile: split the copy across both idle engines
                        nc.scalar.copy(osb[:, 0:512], fin[:, 0, :])
                        nc.vector.tensor_copy(osb[:, 512:1024], fin[:, 1, :])
                    elif j % 2 == 0:
                        nc.scalar.copy(osb[:], fin[:])
                    else:
                        nc.vector.tensor_copy(osb[:], fin[:])
                    nc.sync.dma_start(
                        out_d[qt_i * P : (qt_i + 1) * P, :], osb[:]
                    )

            phase_a0()
            nc.sync.dma_start(wo_sb[:], wo_d.rearrange("(c p) m -> p c m", p=P))
            load_inputs(1)
            for qb in range(NQB):
                if qb + 1 < NQB:
                    groups = phase_a_groups(qb + 1)
                    phase_b(qb, filler=groups)
                    if qb + 2 < NQB:
                        load_inputs(qb + 2)
                    for g in groups:
                        g()
                else:
                    phase_b(qb)
                phase_c(qb)

    nc.compile()
    return nc


def _get_program(kcap=NT):
    key = kcap
    if key not in _CACHE:
        _CACHE[key] = _build_program(kcap=kcap)
    return _CACHE[key]


def _make_in_maps(q_input, k_input, v_input, key_padding_mask, Wq, Wk, Wv, Wo, bq, bk):
    bf = ml_dtypes.bfloat16
    mask128 = (
        np.arange(P)[None, :] >= np.arange(P)[:, None]
    ).astype(bf)  # keep[k, q] = 1 where key k <= query q within the block
    mask2 = np.ascontiguousarray(
        np.broadcast_to(mask128[:, None, :], (P, 2, P))
    )
    in_maps = []
    for core in range(8):
        b = core // 2
        hg = core % 2
        sl = slice(hg * DHH, (hg + 1) * DHH)
        padv = np.where(key_padding_mask[b], NEG_PAD, 0.0).astype(np.float32)
        in_maps.append(
            {
                "qt": np.ascontiguousarray(q_input[b].astype(bf).T),
                "kt": np.ascontiguousarray(k_input[b].astype(bf).T),
                "vt": np.ascontiguousarray(v_input[b].astype(bf).T),
                "wq": Wq[:, sl].astype(bf),
                "wk": Wk[:, sl].astype(bf),
                "wv": Wv[:, sl].astype(bf),
                "wo": np.ascontiguousarray(Wo[sl, :]).astype(bf),
                "padb": np.ascontiguousarray(padv.reshape(NT, P).T),
                "mask": mask2,
                "bq": np.ascontiguousarray(bq[sl].reshape(4, P).T.astype(np.float32)),
                "bk": np.ascontiguousarray(bk[sl].reshape(4, P).T.astype(np.float32)),
            }
        )
    return in_maps


def run_spmd(in_maps, kcap=NT, **kwargs):
    from concourse import bass_utils

    nc = _get_program(kcap=kcap)
    return bass_utils.run_bass_kernel_spmd(
        nc, in_maps, core_ids=list(range(8)), **kwargs
    )


def kernel(q_input, k_input, v_input, key_padding_mask,
           Wq, bq, Wk, bk, Wv, bv, Wo, bo, **_unused):
    q_input = np.asarray(q_input, dtype=np.float32)
    k_input = np.asarray(k_input, dtype=np.float32)
    v_input = np.asarray(v_input, dtype=np.float32)
    key_padding_mask = np.asarray(key_padding_mask)
    in_maps = _make_in_maps(
        q_input, k_input, v_input, key_padding_mask,
        np.asarray(Wq, np.float32), np.asarray(Wk, np.float32),
        np.asarray(Wv, np.float32), np.asarray(Wo, np.float32),
        np.asarray(bq, np.float32), np.asarray(bk, np.float32),
    )
    valid = S - key_padding_mask.astype(np.int64).sum(axis=1)
    kcap = int(min(NT, max(1, -(-int(valid.max()) // P))))
    res = run_spmd(in_maps, kcap=kcap).results
    bo = np.asarray(bo, np.float32)
    bv = np.asarray(bv, np.float32)
    # bv support: normalized attention plus bv equals attn output with biased V
    # (rows of softmax sum to 1) -> fold bv through Wo into the output bias.
    extra = bv @ np.asarray(Wo, np.float32) if np.any(bv) else 0.0
    out = np.stack(
        [
            res[2 * b]["outp"].astype(np.float32)
            + res[2 * b + 1]["outp"].astype(np.float32)
            for b in range(4)
        ]
    ) + bo + extra
    return out.astype(np.float32)


# revision 57
# speedup vs baseline: 1.0505x; 1.0505x over previous
"""Trainium2 Bass kernel for nn_AttentionUnit (B=4, S=2048, D=1024, H=16).

Sharding: 8 cores = 4 batches x 2 head-groups (Megatron column/row split).
Each core computes, for its (batch, 8-head half):
  Q^T,K^T = (Wq/Wk half)^T-proj of inputs   [dh=512 on partitions, seq free]
  V       = natural [seq, dh] per head, with a ones column on each side
  S^T     = K @ Q^T / 8 per head-pair tile (causal blocks skipped,
            padding via exp bias shared across the pair)
  P^T     = exp(S^T)  (unnormalized, bf16, one activation per (pair, kt))
  O^T     = V_aug^T @ P^T  -> even heads rows 0..64 (denom at 64),
            odd heads rows 63..127 (denom at 63): attn lands at its final
            partition offset, no cross-partition DMA.
  attn^T  = O^T * recip(denom) -> AT [512, seq] bf16 on chip
  partial = attn @ Wo_half     [seq, 1024] fp32 -> host (direct PSUM->DRAM)
Host: out[b] = partial[2b] + partial[2b+1] + bo (+ bv@Wo).

Schedule: warmup matmuls ramp the PE clock during the initial weight DMA;
per q-block emission order is B(qb), A(qb+1), C(qb) so the output
projection never waits on the last head-pair's normalize chain.
"""

import sys

sys.path.insert(0, "/opt/trn_rl_repo")

import numpy as np
import ml_dtypes

S = 2048
D = 1024
P = 128
DH = 64          # head dim
HPC = 8          # heads per core
DHH = 512        # dh per core (8 heads * 64)
QB = 512         # q block
NQB = S // QB    # 4
DMC = D // P     # 8 dmodel chunks
NT = S // P      # 16 k tiles
NEG_CAUSAL = -1.0e12   # added pre-scale (scale=0.125 applied inside exp)
NEG_PAD = -1.0e9       # added post-scale (exp bias)
NWARM = 5              # PE clock-ramp matmuls during initial DMA

_CACHE = {}


def _build_program(kcap=NT, warm=NWARM):
    import concourse.bass as bass
    import concourse.tile as tile
    from concourse import bacc, mybir

    f32 = mybir.dt.float32
    bf16 = mybir.dt.bfloat16
    ADD = mybir.AluOpType.add
    MUL = mybir.AluOpType.mult
    EXP = mybir.ActivationFunctionType.Exp

    nc = bacc.Bacc("TRN2", target_bir_lowering=False, debug=False)

    qt_d = nc.dram_tensor("qt", [D, S], bf16, kind="ExternalInput")
    kt_d = nc.dram_tensor("kt", [D, S], bf16, kind="ExternalInput")
    vt_d = nc.dram_tensor("vt", [D, S], bf16, kind="ExternalInput")
    wq_d = nc.dram_tensor("wq", [D, DHH], bf16, kind="ExternalInput")
    wk_d = nc.dram_tensor("wk", [D, DHH], bf16, kind="ExternalInput")
    wv_d = nc.dram_tensor("wv", [D, DHH], bf16, kind="ExternalInput")
    wo_d = nc.dram_tensor("wo", [DHH, D], bf16, kind="ExternalInput")
    padb_d = nc.dram_tensor("padb", [P, NT], f32, kind="ExternalInput")
    mask_d = nc.dram_tensor("mask", [P, 2, P], bf16, kind="ExternalInput")
    bq_d = nc.dram_tensor("bq", [P, 4], f32, kind="ExternalInput")
    bk_d = nc.dram_tensor("bk", [P, 4], f32, kind="ExternalInput")
    out_d = nc.dram_tensor("outp", [S, D], bf16, kind="ExternalOutput")

    qt_r = qt_d.rearrange("(c p) s -> p c s", p=P)
    kt_r = kt_d.rearrange("(c p) s -> p c s", p=P)
    vt_r = vt_d.rearrange("(c p) s -> p c s", p=P)

    with tile.TileContext(nc) as tc:
        with (
            tc.tile_pool(name="const", bufs=1) as constp,
            tc.tile_pool(name="inp", bufs=2) as inp,
            tc.tile_pool(name="probs", bufs=4) as probsp,
            tc.tile_pool(name="small", bufs=2) as smallp,
            tc.tile_pool(name="pvs", bufs=2) as pvsp,
            tc.tile_pool(name="osb", bufs=2) as osbp,
            tc.tile_pool(name="psSC", bufs=3, space="PSUM") as psSC,
            tc.tile_pool(name="psPV", bufs=2, space="PSUM") as psPV,
        ):
            # ---- warmup: ramp the PE clock while weights stream in ----
            wa = constp.tile([P, P], bf16, tag="wa")
            wb = constp.tile([P, QB], bf16, tag="wb")
            nc.vector.memset(wa[:], 0.0)
            nc.vector.memset(wb[:], 0.0)
            wps = psSC.tile([P, 2, QB], f32, tag="sc")
            for i in range(warm):
                nc.tensor.matmul(wps[:, 0, :], wa[:], wb[:], start=True, stop=True)
                if i == 1:
                    # pull the Exp table load off the critical path
                    wact = constp.tile([P, DH], bf16, tag="wact")
                    nc.scalar.activation(wact[:], wps[:, 0, 0:DH], EXP, scale=1.0)

            # ---- persistent SBUF tensors ----
            wq_sb = constp.tile([P, DMC, DHH], bf16, tag="wq")
            wk_sb = constp.tile([P, DMC, DHH], bf16, tag="wk")
            wv_sb = constp.tile([P, DMC, DHH], bf16, tag="wv")
            wo_sb = constp.tile([P, 4, D], bf16, tag="wo")
            QT = constp.tile([P, 4, S], bf16, tag="QT")
            KT = constp.tile([P, 4, S], bf16, tag="KT")
            V = constp.tile([P, NT, HPC, DH + 1], bf16, tag="V")
            AT = constp.tile([P, 4, S], bf16, tag="AT")
            padb = constp.tile([P, NT], f32, tag="padb")
            mask = constp.tile([P, 2, P], bf16, tag="mask")
            bq_sb = constp.tile([P, 4], f32, tag="bq")
            bk_sb = constp.tile([P, 4], f32, tag="bk")

            # qb0 inputs and weights stream in matched mc-chunks, with the
            # accumulating projection matmuls emitted right behind each chunk
            # pair so the PE starts after ~300KB instead of the full 8MB.
            wq_r = wq_d.rearrange("(c p) m -> p c m", p=P)
            wk_r = wk_d.rearrange("(c p) m -> p c m", p=P)
            wv_r = wv_d.rearrange("(c p) m -> p c m", p=P)
            qin0 = inp.tile([P, DMC, QB], bf16, tag="qin")
            kin0 = inp.tile([P, DMC, QB], bf16, tag="kin")
            vin0 = inp.tile([P, DMC, QB], bf16, tag="vin")
            nc.sync.dma_start(padb[:], padb_d[:])
            nc.sync.dma_start(mask[:], mask_d[:])
            nc.sync.dma_start(bq_sb[:], bq_d[:])
            nc.sync.dma_start(bk_sb[:], bk_d[:])
            # ones column for the softmax denominator
            nc.vector.memset(V[:, :, :, DH : DH + 1], 1.0)

            ins = [None] * (NQB + 1)
            ins[0] = (qin0, kin0, vin0)

            def stream_proj(w_sb, w_r, in_sb, in_r, operands_of_mc):
                """DMA-chase projection: per-mc chunk DMAs immediately
                followed by the 4 accumulating matmuls that consume them.
                Returns the two PSUM tiles holding c-chunks (0,1) and (2,3)."""
                t01 = psSC.tile([P, 2, QB], f32, tag="sc")
                t23 = psSC.tile([P, 2, QB], f32, tag="sc")
                tiles = [t01, t23]
                for mc in range(DMC):
                    nc.sync.dma_start(w_sb[:, mc, :], w_r[:, mc, :])
                    nc.sync.dma_start(in_sb[:, mc, :], in_r[:, mc, 0:QB])
                    for c in range(4):
                        lhsT, rhs = operands_of_mc(w_sb, in_sb, mc, c)
                        nc.tensor.matmul(
                            tiles[c // 2][:, c % 2, :], lhsT, rhs,
                            start=(mc == 0), stop=(mc == DMC - 1),
                        )
                return tiles

            def phase_a0():
                tq = stream_proj(
                    wq_sb, wq_r, qin0, qt_r,
                    lambda w, i, mc, c: (w[:, mc, c * P : (c + 1) * P], i[:, mc, :]),
                )
                for c in range(4):
                    nc.scalar.add(
                        QT[:, c, 0:QB], tq[c // 2][:, c % 2, :], bq_sb[:, c : c + 1]
                    )
                tk = stream_proj(
                    wk_sb, wk_r, kin0, kt_r,
                    lambda w, i, mc, c: (w[:, mc, c * P : (c + 1) * P], i[:, mc, :]),
                )
                for c in range(4):
                    nc.scalar.add(
                        KT[:, c, 0:QB], tk[c // 2][:, c % 2, :], bk_sb[:, c : c + 1]
                    )
                tv = stream_proj(
                    wv_sb, wv_r, vin0, vt_r,
                    lambda w, i, mc, j: (i[:, mc, j * P : (j + 1) * P], w[:, mc, :]),
                )
                for j in range(4):
                    nc.vector.tensor_copy(
                        V[:, j, :, 0:DH],
                        tv[j // 2][:, j % 2, :].rearrange("p (h d) -> p h d", d=DH),
                    )

            def load_inputs(qb):
                q0 = qb * QB
                qin = inp.tile([P, DMC, QB], bf16, tag="qin")
                kin = inp.tile([P, DMC, QB], bf16, tag="kin")
                vin = inp.tile([P, DMC, QB], bf16, tag="vin")
                nc.sync.dma_start(qin[:], qt_r[:, :, q0 : q0 + QB])
                nc.sync.dma_start(kin[:], kt_r[:, :, q0 : q0 + QB])
                nc.sync.dma_start(vin[:], vt_r[:, :, q0 : q0 + QB])
                ins[qb] = (qin, kin, vin)

            def phase_a_groups(qb):
                """Projection work for q-block qb as 6 emission closures
                [Q01, K01, Q23, K23, V01, V23] so groups can be interleaved
                at B(qb-1) pair boundaries as PE filler."""
                q0 = qb * QB
                qin, kin, vin = ins[qb]
                kn = min(QB, max(0, kcap * P - q0))

                def qgrp(cp):
                    ps = psSC.tile([P, 2, QB], f32, tag="sc")
                    for jj in range(2):
                        c = 2 * cp + jj
                        for mc in range(DMC):
                            nc.tensor.matmul(
                                ps[:, jj, :],
                                wq_sb[:, mc, c * P : (c + 1) * P],
                                qin[:, mc, :],
                                start=(mc == 0),
                                stop=(mc == DMC - 1),
                            )
                    for jj in range(2):
                        c = 2 * cp + jj
                        nc.scalar.add(
                            QT[:, c, q0 : q0 + QB], ps[:, jj, :],
                            bq_sb[:, c : c + 1],
                        )

                def kgrp(cp):
                    if kn <= 0:
                        return
                    ps = psSC.tile([P, 2, QB], f32, tag="sc")
                    for jj in range(2):
                        c = 2 * cp + jj
                        for mc in range(DMC):
                            nc.tensor.matmul(
                                ps[:, jj, :kn],
                                wk_sb[:, mc, c * P : (c + 1) * P],
                                kin[:, mc, :kn],
                                start=(mc == 0),
                                stop=(mc == DMC - 1),
                            )
                    for jj in range(2):
                        c = 2 * cp + jj
                        nc.scalar.add(
                            KT[:, c, q0 : q0 + kn], ps[:, jj, :kn],
                            bk_sb[:, c : c + 1],
                        )

                def vgrp(jp):
                    if 4 * qb + 2 * jp >= kcap:
                        return
                    ps = psSC.tile([P, 2, QB], f32, tag="sc")
                    for jj in range(2):
                        j = 2 * jp + jj
                        if 4 * qb + j >= kcap:
                            continue
                        for mc in range(DMC):
                            nc.tensor.matmul(
                                ps[:, jj, :],
                                vin[:, mc, j * P : (j + 1) * P],
                                wv_sb[:, mc, :],
                                start=(mc == 0),
                                stop=(mc == DMC - 1),
                            )
                    for jj in range(2):
                        j = 2 * jp + jj
                        kt_i = 4 * qb + j
                        if kt_i >= kcap:
                            continue
                        nc.vector.tensor_copy(
                            V[:, kt_i, :, 0:DH],
                            ps[:, jj, :].rearrange("p (h d) -> p h d", d=DH),
                        )

                return [
                    lambda: qgrp(0), lambda: kgrp(0),
                    lambda: qgrp(1), lambda: kgrp(1),
                    lambda: vgrp(0), lambda: vgrp(1),
                ]

            def phase_b(qb, filler=None):
                """Attention for q-block qb -> AT[:, :, q0:q0+QB]. After
                pairs 0-2, one next-block projection group is emitted as PE
                filler to cover the normalize-chain drain on DVE."""
                q0 = qb * QB
                kt_max = min(4 * qb + 4, kcap)
                for pair in range(4):
                    h0, h1 = 2 * pair, 2 * pair + 1
                    pv_e = psPV.tile([P, QB], f32, tag="pv")
                    pv_o = psPV.tile([P, QB], f32, tag="pv")

                    def emit_sc(kt):
                        delta = max(0, P * kt - q0)
                        sc = psSC.tile([P, 2, QB], f32, tag="sc")
                        nc.tensor.matmul(
                            sc[:, 0, delta:],
                            KT[0:DH, pair, kt * P : (kt + 1) * P],
                            QT[0:DH, pair, q0 + delta : q0 + QB],
                            start=True, stop=True,
                        )
                        nc.tensor.matmul(
                            sc[:, 1, delta:],
                            KT[DH:P, pair, kt * P : (kt + 1) * P],
                            QT[DH:P, pair, q0 + delta : q0 + QB],
                            start=True, stop=True,
                        )
                        return sc

                    def emit_exp(kt, sc):
                        delta = max(0, P * kt - q0)
                        pr = probsp.tile([P, 2, QB], bf16, tag="probs")
                        nc.scalar.activation(
                            pr[:, :, delta:],
                            sc[:, :, delta:],
                            EXP,
                            bias=padb[:, kt : kt + 1],
                            scale=0.125,
                        )
                        if kt >= 4 * qb:
                            # zero the upper-triangle probs of the diagonal
                            # block (cheap bf16 SBUF multiply, 2x DVE mode)
                            nc.vector.tensor_tensor(
                                pr[:, :, delta : delta + P],
                                pr[:, :, delta : delta + P],
                                mask[:],
                                MUL,
                            )
                        return pr

                    def emit_pv(kt, pr):
                        delta = max(0, P * kt - q0)
                        nc.tensor.matmul(
                            pv_e[0 : DH + 1, delta:],
                            V[:, kt, h0, 0 : DH + 1],
                            pr[:, 0, delta:],
                            start=(kt == 0),
                            stop=(kt == kt_max - 1),
                        )
                        nc.tensor.matmul(
                            pv_o[0 : DH + 1, delta:],
                            V[:, kt, h1, 0 : DH + 1],
                            pr[:, 1, delta:],
                            start=(kt == 0),
                            stop=(kt == kt_max - 1),
                        )

                    # 2-deep software pipeline: the PE stream runs sc(k+2)
                    # ahead of pv(k) so pv's exp wait (and the pv-bank WAR at
                    # pair transitions) is fully covered.
                    prs = {}
                    for kt in range(min(2, kt_max)):
                        prs[kt] = emit_exp(kt, emit_sc(kt))
                    for kt in range(kt_max):
                        if kt + 2 < kt_max:
                            sc = emit_sc(kt + 2)
                            emit_pv(kt, prs.pop(kt))
                            prs[kt + 2] = emit_exp(kt + 2, sc)
                        else:
                            emit_pv(kt, prs.pop(kt))
                    # normalize: copy PSUM out early (releases pv banks), DMA
                    # the two denom rows to lane 0, one approx recip each, two
                    # partition broadcasts, two multiplies; odd half reaches
                    # its upper partitions via one SBUF->SBUF DMA. The very
                    # last pair is chunked so phase C's first output tile can
                    # start as soon as its 128 columns are normalized.
                    cp_e = pvsp.tile([P, QB], f32, tag="pvc")
                    cp_o = pvsp.tile([P, QB], f32, tag="pvc")
                    last = (qb == NQB - 1) and (pair == 3)
                    chunks = (P, P, P, P) if last else (QB,)
                    c0 = 0
                    for w in chunks:
                        nc.vector.tensor_copy(
                            cp_e[0 : DH + 1, c0 : c0 + w], pv_e[0 : DH + 1, c0 : c0 + w]
                        )
                        nc.vector.tensor_copy(
                            cp_o[0 : DH + 1, c0 : c0 + w], pv_o[0 : DH + 1, c0 : c0 + w]
                        )
                        c0 += w
                    x0 = 0
                    for w in chunks:
                        xs = slice(x0, x0 + w)
                        den_e = smallp.tile([1, QB], f32, tag="dene")
                        den_o = smallp.tile([1, QB], f32, tag="deno")
                        nc.sync.dma_start(den_e[0:1, 0:w], cp_e[DH : DH + 1, xs])
                        nc.sync.dma_start(den_o[0:1, 0:w], cp_o[DH : DH + 1, xs])
                        rcp_e = smallp.tile([1, QB], f32, tag="rcpe")
                        rcp_o = smallp.tile([1, QB], f32, tag="rcpo")
                        nc.vector.reciprocal_approx_fast(
                            rcp_e[0:1, 0:w], den_e[0:1, 0:w]
                        )
                        nc.vector.reciprocal_approx_fast(
                            rcp_o[0:1, 0:w], den_o[0:1, 0:w]
                        )
                        rep = smallp.tile([DH, 2, QB], f32, tag="rep")
                        nc.gpsimd.partition_broadcast(rep[:, 0, 0:w], rcp_e[0:1, 0:w])
                        nc.gpsimd.partition_broadcast(rep[:, 1, 0:w], rcp_o[0:1, 0:w])
                        nc.vector.tensor_tensor(
                            AT[0:DH, pair, q0 + x0 : q0 + x0 + w],
                            cp_e[0:DH, xs], rep[:, 0, 0:w], MUL,
                        )
                        tmp = smallp.tile([DH, QB], bf16, tag="tmp")
                        nc.vector.tensor_tensor(
                            tmp[0:DH, 0:w], cp_o[0:DH, xs], rep[:, 1, 0:w], MUL,
                        )
                        nc.sync.dma_start(
                            AT[DH:P, pair, q0 + x0 : q0 + x0 + w], tmp[0:DH, 0:w]
                        )
                        x0 += w

            def phase_c(qb):
                """Output projection for q-block qb -> out rows."""
                for j in range(4):
                    qt_i = 4 * qb + j
                    fin = psSC.tile([P, 2, QB], f32, tag="sc")
                    for half in range(2):
                        for c in range(4):
                            nc.tensor.matmul(
                                fin[:, half, :],
                                AT[:, c, qt_i * P : (qt_i + 1) * P],
                                wo_sb[:, c, half * 512 : half * 512 + 512],
                                start=(c == 0),
                                stop=(c == 3),
                            )
                    osb = osbp.tile([P, D], bf16, tag="osb")
                    if qb == NQB - 1 and j == 3:
                        # final tile: split the copy across both idle engines
                        nc.scalar.copy(osb[:, 0:512], fin[:, 0, :])
                        nc.vector.tensor_copy(osb[:, 512:1024], fin[:, 1, :])
                    elif j % 2 == 0:
                        nc.scalar.copy(osb[:], fin[:])
                    else:
                        nc.vector.tensor_copy(osb[:], fin[:])
                    nc.sync.dma_start(
                        out_d[qt_i * P : (qt_i + 1) * P, :], osb[:]
                    )

            phase_a0()
            nc.sync.dma_start(wo_sb[:], wo_d.rearrange("(c p) m -> p c m", p=P))
            load_inputs(1)
            for qb in range(NQB):
                if qb + 1 < NQB:
                    groups = phase_a_groups(qb + 1)
                    phase_b(qb, filler=groups)
                    if qb + 2 < NQB:
                        load_inputs(qb + 2)
                    for g in groups:
                        g()
                else:
                    phase_b(qb)
                phase_c(qb)

    nc.compile()
    return nc


def _get_program(kcap=NT):
    key = kcap
    if key not in _CACHE:
        _CACHE[key] = _build_program(kcap=kcap)
    return _CACHE[key]


def _make_in_maps(q_input, k_input, v_input, key_padding_mask, Wq, Wk, Wv, Wo, bq, bk):
    bf = ml_dtypes.bfloat16
    mask128 = (
        np.arange(P)[None, :] >= np.arange(P)[:, None]
    ).astype(bf)  # keep[k, q] = 1 where key k <= query q within the block
    mask2 = np.ascontiguousarray(
        np.broadcast_to(mask128[:, None, :], (P, 2, P))
    )
    in_maps = []
    for core in range(8):
        b = core // 2
        hg = core % 2
        sl = slice(hg * DHH, (hg + 1) * DHH)
        padv = np.where(key_padding_mask[b], NEG_PAD, 0.0).astype(np.float32)
        in_maps.append(
            {
                "qt": np.ascontiguousarray(q_input[b].astype(bf).T),
                "kt": np.ascontiguousarray(k_input[b].astype(bf).T),
                "vt": np.ascontiguousarray(v_input[b].astype(bf).T),
                "wq": Wq[:, sl].astype(bf),
                "wk": Wk[:, sl].astype(bf),
                "wv": Wv[:, sl].astype(bf),
                "wo": np.ascontiguousarray(Wo[sl, :]).astype(bf),
                "padb": np.ascontiguousarray(padv.reshape(NT, P).T),
                "mask": mask2,
                "bq": np.ascontiguousarray(bq[sl].reshape(4, P).T.astype(np.float32)),
                "bk": np.ascontiguousarray(bk[sl].reshape(4, P).T.astype(np.float32)),
            }
        )
    return in_maps


def run_spmd(in_maps, kcap=NT, **kwargs):
    from concourse import bass_utils

    nc = _get_program(kcap=kcap)
    return bass_utils.run_bass_kernel_spmd(
        nc, in_maps, core_ids=list(range(8)), **kwargs
    )


def kernel(q_input, k_input, v_input, key_padding_mask,
           Wq, bq, Wk, bk, Wv, bv, Wo, bo, **_unused):
    q_input = np.asarray(q_input, dtype=np.float32)
    k_input = np.asarray(k_input, dtype=np.float32)
    v_input = np.asarray(v_input, dtype=np.float32)
    key_padding_mask = np.asarray(key_padding_mask)
    in_maps = _make_in_maps(
        q_input, k_input, v_input, key_padding_mask,
        np.asarray(Wq, np.float32), np.asarray(Wk, np.float32),
        np.asarray(Wv, np.float32), np.asarray(Wo, np.float32),
        np.asarray(bq, np.float32), np.asarray(bk, np.float32),
    )
    valid = S - key_padding_mask.astype(np.int64).sum(axis=1)
    kcap = int(min(NT, max(1, -(-int(valid.max()) // P))))
    res = run_spmd(in_maps, kcap=kcap).results
    bo = np.asarray(bo, np.float32)
    bv = np.asarray(bv, np.float32)
    # bv support: normalized attention plus bv equals attn output with biased V
    # (rows of softmax sum to 1) -> fold bv through Wo into the output bias.
    extra = bv @ np.asarray(Wo, np.float32) if np.any(bv) else 0.0
    out = np.stack(
        [
            res[2 * b]["outp"].astype(np.float32)
            + res[2 * b + 1]["outp"].astype(np.float32)
            for b in range(4)
        ]
    ) + bo + extra
    return out.astype(np.float32)
